# revision 10
# baseline (speedup 1.0000x reference)
"""Trainium2 Bass kernel for nn_Encoder (conv stack + VQ codebook).

Reference computation (fp32):
  x = mels [32, 80, 1024]
  5x (conv1d + batchnorm-affine + relu), 1x 1x1-conv + bias  -> z [32, 64, 511]
  VQ: nearest codebook row (squared L2, 512 codes, D=64) -> q_st, loss, perplexity

Sharding: data-parallel over batch across 8 NeuronCores (4 batches/core);
conv weights + codebook replicated.

Conv matmuls use a bf16 hi/lo 3-term decomposition (W=Wh+Wl, X=Xh+Xl;
Y ~= Wh.Xh + Wh.Xl + Wl.Xh) which runs at 1 cycle/row on the PE instead of
fp32's 4, with ~1.3e-5 relative output error (verified: zero VQ argmin flips
vs the fp32 reference).  Activations are stored in DRAM as pre-split bf16
hi/lo pairs.  The VQ distance computation stays fully fp32 and mirrors the
reference's operation order.  Host gathers z/idx and finishes the (tiny)
statistics exactly as the reference does.
"""

import numpy as np

import concourse.bass as bass
import concourse.tile as tile
from concourse import bacc, mybir
from concourse.bass_utils import run_bass_kernel_spmd

N_CORES = 8
B_LOC = 4            # batches per core
CIN = 80
C = 768
D = 64
M = 512              # codebook size
T0 = 1024            # input T
T2 = 511             # after stride-2 conv3 (through conv6)
NCH = C // 128       # 6 channel chunks
F32 = mybir.dt.float32
FP16 = mybir.dt.float16
U32 = mybir.dt.uint32
LO_SCALE = np.float32(2048.0)

_CACHED_NC = None


def _build_nc():
    nc = bacc.Bacc("TRN2", target_bir_lowering=False, debug=False,
                   num_devices=N_CORES)

    # ---- I/O -----------------------------------------------------------
    mels_h = nc.dram_tensor("mels_h", [B_LOC, CIN, T0], FP16, kind="ExternalInput")
    mels_l = nc.dram_tensor("mels_l", [B_LOC, CIN, T0], FP16, kind="ExternalInput")
    wts = {}
    for li, K, cin in ((1, 3, CIN), (2, 3, C), (3, 4, C), (4, 3, C), (5, 3, C)):
        for p in ("h", "l"):
            wts[li, p] = nc.dram_tensor(f"w{li}T{p}", [K, cin, C], FP16,
                                        kind="ExternalInput")
    for p in ("h", "l"):
        wts[6, p] = nc.dram_tensor(f"w6T{p}", [C, D], FP16, kind="ExternalInput")
    bnS = nc.dram_tensor("bnS", [5, NCH, 128, 1], F32, kind="ExternalInput")
    bnB = nc.dram_tensor("bnB", [5, NCH, 128, 1], F32, kind="ExternalInput")
    b6v = nc.dram_tensor("b6v", [D, 1], F32, kind="ExternalInput")
    embT2 = nc.dram_tensor("embT2", [D, M], F32, kind="ExternalInput")   # 2*emb.T
    e2n = nc.dram_tensor("e2n", [128, M], F32, kind="ExternalInput")     # -|e|^2 bcast

    z_out = nc.dram_tensor("z_out", [B_LOC, D, T2], F32, kind="ExternalOutput")
    idx_out = nc.dram_tensor("idx_out", [128, 16], U32, kind="ExternalOutput")

    with tile.TileContext(nc) as tc:
        with (
            tc.tile_pool(name="const", bufs=1) as constp,
            tc.tile_pool(name="dram", bufs=1, space="DRAM") as dramp,
        ):
            # ---- constants ------------------------------------------------
            bn_s = {}
            bn_b = {}
            for li in range(5):
                for ci in range(NCH):
                    s = constp.tile([128, 1], F32, tag=f"bns_{li}_{ci}")
                    b = constp.tile([128, 1], F32, tag=f"bnb_{li}_{ci}")
                    nc.sync.dma_start(out=s, in_=bnS[li, ci, :, :])
                    nc.sync.dma_start(out=b, in_=bnB[li, ci, :, :])
                    bn_s[li, ci] = s
                    bn_b[li, ci] = b
            b6s = constp.tile([D, 1], F32, tag="b6s")
            nc.sync.dma_start(out=b6s, in_=b6v[:, :])
            embT2s = constp.tile([D, M], F32, tag="embT2s")
            nc.sync.dma_start(out=embT2s, in_=embT2[:, :])
            e2ns = constp.tile([128, M], F32, tag="e2ns")
            nc.sync.dma_start(out=e2ns, in_=e2n[:, :])
            ones64 = constp.tile([D, 1], F32, tag="ones64")
            nc.vector.memset(ones64, 1.0)
            idxacc = constp.tile([128, 16], U32, tag="idxacc")

            # ---- DRAM scratch: bf16 hi/lo activation pairs ----------------
            act = {}
            for li, width in ((1, T0), (2, T0), (3, 513), (4, 513), (5, T2)):
                for b in range(B_LOC):
                    for ci in range(NCH):
                        for p in ("h", "l"):
                            act[li, b, ci, p] = dramp.tile(
                                [128, width], FP16, tag=f"a{li}_{b}_{ci}{p}",
                                name=f"a{li}_{b}_{ci}{p}")

            def load_weights(wp, li, K, cin_p, n_cin):
                wt = {}
                for p in ("h", "l"):
                    for k in range(K):
                        for ci in range(n_cin):
                            t = wp.tile([cin_p, C], FP16, tag=f"w{li}{p}_{k}_{ci}",
                                        name=f"w{li}{p}_{k}_{ci}")
                            nc.sync.dma_start(
                                out=t,
                                in_=wts[li, p][k, ci * cin_p:(ci + 1) * cin_p, :])
                            wt[p, k, ci] = t
                return wt

            def conv_layer(li, K, tiles_spec, get_in, wt, n_cin, psump,
                           conv_in, conv_out):
                cin_p = wt["h", 0, 0].shape[0]
                for b in range(B_LOC):
                    for (t0, tw, in_lo, in_w, out_lo, pad_lo, pad_hi,
                         stride) in tiles_spec:
                        ins = {}
                        for ci in range(n_cin):
                            for p in ("h", "l"):
                                it = conv_in.tile([cin_p, in_w], FP16,
                                                  tag=f"in{ci}{p}", bufs=2,
                                                  name=f"cin{li}_{ci}{p}")
                                nc.sync.dma_start(
                                    out=it, in_=get_in(b, ci, p, in_lo, in_w))
                                ins[ci, p] = it
                        for co in range(NCH):
                            psA = psump.tile([128, tw], F32, tag="cpsA")
                            psB = psump.tile([128, tw], F32, tag="cpsB")
                            nA = n_cin * K
                            i = 0
                            for ci in range(n_cin):
                                for k in range(K):
                                    if stride == 1:
                                        sl = slice(k, k + tw)
                                    else:
                                        sl = slice(k, k + 2 * (tw - 1) + 1, 2)
                                    wh = wt["h", k, ci][:, co * 128:(co + 1) * 128]
                                    nc.tensor.matmul(psA[:, :], wh,
                                                     ins[ci, "h"][:, sl],
                                                     start=(i == 0),
                                                     stop=(i == nA - 1))
                                    i += 1
                            i = 0
                            for ci in range(n_cin):
                                for k in range(K):
                                    if stride == 1:
                                        sl = slice(k, k + tw)
                                    else:
                                        sl = slice(k, k + 2 * (tw - 1) + 1, 2)
                                    wh = wt["h", k, ci][:, co * 128:(co + 1) * 128]
                                    wl = wt["l", k, ci][:, co * 128:(co + 1) * 128]
                                    for lhsT, rhs in ((wh, ins[ci, "l"][:, sl]),
                                                      (wl, ins[ci, "h"][:, sl])):
                                        nc.tensor.matmul(psB[:, :], lhsT, rhs,
                                                         start=(i == 0),
                                                         stop=(i == 2 * nA - 1))
                                        i += 1
                            tlo = conv_out.tile([128, tw], F32, tag="tlo",
                                                bufs=3, name=f"tlo{li}_{co}")
                            nc.vector.tensor_scalar_mul(tlo[:, :], psB[:, :],
                                                        1.0 / 2048.0)
                            pre = conv_out.tile([128, tw], F32, tag="pre",
                                                bufs=3, name=f"pre{li}_{co}")
                            nc.vector.tensor_add(pre[:, :], tlo[:, :], psA[:, :])
                            y32 = conv_out.tile([128, tw], F32, tag="y",
                                                bufs=3, name=f"y{li}_{co}")
                            nc.scalar.activation(
                                out=y32[:, :], in_=pre[:, :],
                                func=mybir.ActivationFunctionType.Relu,
                                bias=bn_b[li - 1, co][:, :],
                                scale=bn_s[li - 1, co][:, :])
                            w_out = pad_lo + tw + pad_hi
                            for p in ("h", "l"):
                                ot = conv_out.tile([128, w_out], FP16,
                                                   tag=f"o{p}", bufs=4,
                                                   name=f"o{li}_{co}{p}")
                                if pad_lo:
                                    nc.vector.memset(ot[:, 0:pad_lo], 0.0)
                                if pad_hi:
                                    nc.vector.memset(
                                        ot[:, pad_lo + tw:w_out], 0.0)
                                if p == "h":
                                    nc.vector.tensor_copy(
                                        ot[:, pad_lo:pad_lo + tw], y32[:, :])
                                    oh = ot
                                else:
                                    tsub = conv_out.tile(
                                        [128, tw], F32, tag="ts",
                                        bufs=3, name=f"ts{li}_{co}")
                                    nc.vector.tensor_sub(
                                        tsub[:, :], y32[:, :],
                                        oh[:, pad_lo:pad_lo + tw])
                                    nc.vector.tensor_scalar_mul(
                                        ot[:, pad_lo:pad_lo + tw], tsub[:, :],
                                        2048.0)
                                nc.sync.dma_start(
                                    out=act[li, b, co, p][:, out_lo:out_lo + w_out],
                                    in_=ot[:, :])

            # ---- L1: conv(80->768, K=3, valid), T 1024 -> 1022 -------------
            with tc.tile_pool(name="w1p", bufs=1) as wp, \
                 tc.tile_pool(name="c1i", bufs=1) as conv_in, \
                 tc.tile_pool(name="c1o", bufs=1) as conv_out, \
                 tc.tile_pool(name="ps1", bufs=2, space="PSUM") as psump:
                wt = load_weights(wp, 1, 3, CIN, 1)
                spec = [(0, 511, 0, 513, 0, 1, 0, 1),
                        (511, 511, 511, 513, 512, 0, 1, 1)]
                mels_d = {"h": mels_h, "l": mels_l}
                conv_layer(1, 3, spec,
                           lambda b, ci, p, lo, w: mels_d[p][b, :, lo:lo + w],
                           wt, 1, psump, conv_in, conv_out)

            # ---- L2: conv(768->768, K=3, pad 1), T 1022 -> 1022 ------------
            with tc.tile_pool(name="w2p", bufs=1) as wp, \
                 tc.tile_pool(name="c2i", bufs=1) as conv_in, \
                 tc.tile_pool(name="c2o", bufs=1) as conv_out, \
                 tc.tile_pool(name="ps2", bufs=2, space="PSUM") as psump:
                wt = load_weights(wp, 2, 3, 128, NCH)
                spec = [(0, 511, 0, 513, 0, 1, 0, 1),
                        (511, 511, 511, 513, 512, 0, 1, 1)]
                conv_layer(2, 3, spec,
                           lambda b, ci, p, lo, w: act[1, b, ci, p][:, lo:lo + w],
                           wt, NCH, psump, conv_in, conv_out)

            # ---- L3: conv(768->768, K=4, pad 1, stride 2), 1022 -> 511 -----
            with tc.tile_pool(name="w3p", bufs=1) as wp, \
                 tc.tile_pool(name="c3i", bufs=1) as conv_in, \
                 tc.tile_pool(name="c3o", bufs=1) as conv_out, \
                 tc.tile_pool(name="ps3", bufs=2, space="PSUM") as psump:
                wt = load_weights(wp, 3, 4, 128, NCH)
                spec = [(0, 511, 0, 1024, 0, 1, 1, 2)]
                conv_layer(3, 4, spec,
                           lambda b, ci, p, lo, w: act[2, b, ci, p][:, lo:lo + w],
                           wt, NCH, psump, conv_in, conv_out)

            # ---- L4: conv(768->768, K=3, pad 1), 511 -> 511 ----------------
            with tc.tile_pool(name="w4p", bufs=1) as wp, \
                 tc.tile_pool(name="c4i", bufs=1) as conv_in, \
                 tc.tile_pool(name="c4o", bufs=1) as conv_out, \
                 tc.tile_pool(name="ps4", bufs=2, space="PSUM") as psump:
                wt = load_weights(wp, 4, 3, 128, NCH)
                spec = [(0, 511, 0, 513, 0, 1, 1, 1)]
                conv_layer(4, 3, spec,
                           lambda b, ci, p, lo, w: act[3, b, ci, p][:, lo:lo + w],
                           wt, NCH, psump, conv_in, conv_out)

            # ---- L5: conv(768->768, K=3, pad 1), 511 -> 511 (no out pad) ---
            with tc.tile_pool(name="w5p", bufs=1) as wp, \
                 tc.tile_pool(name="c5i", bufs=1) as conv_in, \
                 tc.tile_pool(name="c5o", bufs=1) as conv_out, \
                 tc.tile_pool(name="ps5", bufs=2, space="PSUM") as psump:
                wt = load_weights(wp, 5, 3, 128, NCH)
                spec = [(0, 511, 0, 513, 0, 0, 0, 1)]
                conv_layer(5, 3, spec,
                           lambda b, ci, p, lo, w: act[4, b, ci, p][:, lo:lo + w],
                           wt, NCH, psump, conv_in, conv_out)

            # ---- L6 (1x1 conv + bias) and VQ ------------------------------
            with tc.tile_pool(name="w6p", bufs=1) as wp, \
                 tc.tile_pool(name="c6i", bufs=1) as conv_in, \
                 tc.tile_pool(name="vq", bufs=2) as vqp, \
                 tc.tile_pool(name="vqsmall", bufs=4) as vqsp, \
                 tc.tile_pool(name="ps6", bufs=2, space="PSUM") as psump:
                wt6 = {}
                for p in ("h", "l"):
                    for ci in range(NCH):
                        t = wp.tile([128, D], FP16, tag=f"w6{p}_{ci}",
                                    name=f"w6{p}_{ci}")
                        nc.sync.dma_start(
                            out=t, in_=wts[6, p][ci * 128:(ci + 1) * 128, :])
                        wt6[p, ci] = t
                for b in range(B_LOC):
                    ins = {}
                    for ci in range(NCH):
                        for p in ("h", "l"):
                            it = conv_in.tile([128, T2], FP16, tag=f"in{ci}{p}",
                                              bufs=3, name=f"cin6_{ci}{p}")
                            nc.sync.dma_start(out=it, in_=act[5, b, ci, p][:, :])
                            ins[ci, p] = it
                    psA = psump.tile([D, T2], F32, tag="zpsA")
                    psB = psump.tile([D, T2], F32, tag="zpsB")
                    for ci in range(NCH):
                        nc.tensor.matmul(psA[:, :], wt6["h", ci][:, :],
                                         ins[ci, "h"][:, :],
                                         start=(ci == 0), stop=(ci == NCH - 1))
                    i = 0
                    for ci in range(NCH):
                        for lhsT, rhs in ((wt6["h", ci], ins[ci, "l"]),
                                          (wt6["l", ci], ins[ci, "h"])):
                            nc.tensor.matmul(psB[:, :], lhsT[:, :], rhs[:, :],
                                             start=(i == 0),
                                             stop=(i == 2 * NCH - 1))
                            i += 1
                    zlo = vqp.tile([D, T2], F32, tag="zlo")
                    nc.vector.tensor_scalar_mul(zlo[:, :], psB[:, :],
                                                1.0 / 2048.0)
                    zpre = vqp.tile([D, T2], F32, tag="zpre")
                    nc.vector.tensor_add(zpre[:, :], zlo[:, :], psA[:, :])
                    zb = vqp.tile([D, T2], F32, tag="zb")
                    nc.vector.tensor_scalar_add(zb[:, :], zpre[:, :], b6s[:, :])
                    nc.sync.dma_start(out=z_out[b, :, :], in_=zb[:, :])

                    # VQ for this batch (all fp32)
                    zsq = vqp.tile([D, T2], F32, tag="zsq")
                    nc.vector.tensor_mul(zsq[:, :], zb[:, :], zb[:, :])
                    for c in range(4):
                        c0 = c * 128
                        cs = min(128, T2 - c0)
                        x2p = psump.tile([128, 1], F32, tag="x2p")
                        nc.tensor.matmul(x2p[:cs, :], zsq[:, c0:c0 + cs],
                                         ones64[:, :], start=True, stop=True)
                        x2s = vqsp.tile([128, 1], F32, tag="x2s")
                        nc.vector.tensor_copy(x2s[:cs, :], x2p[:cs, :])
                        scp = psump.tile([128, M], F32, tag="scp")
                        nc.tensor.matmul(scp[:cs, :], zb[:, c0:c0 + cs],
                                         embT2s[:, :], start=True, stop=True)
                        # ndist = (-e2 - x2) + 2*dots   == -dists
                        t1 = vqsp.tile([128, M], F32, tag="t1")
                        nc.vector.tensor_scalar_sub(t1[:cs, :], e2ns[:cs, :],
                                                    x2s[:cs, :])
                        nd = vqsp.tile([128, M], F32, tag="nd")
                        nc.vector.tensor_add(nd[:cs, :], t1[:cs, :], scp[:cs, :])
                        mx = vqsp.tile([128, 8], F32, tag="mx")
                        nc.vector.max(mx[:cs, :], nd[:cs, :])
                        mi = vqsp.tile([128, 8], U32, tag="mi")
                        nc.vector.max_index(mi[:cs, :], mx[:cs, :], nd[:cs, :])
                        col = b * 4 + c
                        nc.vector.tensor_copy(idxacc[:cs, col:col + 1],
                                              mi[:cs, 0:1])
            nc.sync.dma_start(out=idx_out[:, :], in_=idxacc[:, :])

    nc.compile()
    return nc


def _get_nc():
    global _CACHED_NC
    if _CACHED_NC is None:
        _CACHED_NC = _build_nc()
    return _CACHED_NC


def _split_hl(x):
    h = x.astype(np.float16)
    l = ((x - h.astype(np.float32)) * LO_SCALE).astype(np.float16)
    return np.ascontiguousarray(h), np.ascontiguousarray(l)


def _host_prep(inputs):
    f = np.float32
    out = {}
    for li, key in ((1, "w1"), (2, "w2"), (3, "w3"), (4, "w4"), (5, "w5")):
        wT = np.ascontiguousarray(inputs[key].astype(f).transpose(2, 1, 0))
        h, l = _split_hl(wT)
        out[f"w{li}Th"] = h
        out[f"w{li}Tl"] = l
    w6 = np.ascontiguousarray(inputs["w6"].astype(f)[:, :, 0].T)
    out["w6Th"], out["w6Tl"] = _split_hl(w6)
    gamma = inputs["bn_gamma"].astype(f)
    beta = inputs["bn_beta"].astype(f)
    mean = inputs["bn_mean"].astype(f)
    var = inputs["bn_var"].astype(f)
    inv = gamma / np.sqrt(var + f(1e-5))
    bias = beta - mean * inv
    out["bnS"] = np.ascontiguousarray(inv.reshape(5, NCH, 128, 1))
    out["bnB"] = np.ascontiguousarray(bias.reshape(5, NCH, 128, 1))
    out["b6v"] = np.ascontiguousarray(inputs["b6"].astype(f).reshape(D, 1))
    emb = inputs["embedding"].astype(f)
    out["embT2"] = np.ascontiguousarray(2.0 * emb.T)
    e2 = np.sum(emb.astype(np.float64) ** 2, axis=1).astype(f)
    out["e2n"] = np.ascontiguousarray(np.broadcast_to(-e2[None, :], (128, M)))
    return out, emb


def _make_in_maps(inputs):
    shared, emb = _host_prep(inputs)
    mels = inputs["mels"].astype(np.float32)
    B = mels.shape[0]
    assert B == N_CORES * B_LOC
    in_maps = []
    for c in range(N_CORES):
        m = dict(shared)
        mh, ml = _split_hl(mels[c * B_LOC:(c + 1) * B_LOC])
        m["mels_h"] = mh
        m["mels_l"] = ml
        in_maps.append(m)
    return in_maps, emb


def kernel(**inputs):
    nc = _get_nc()
    in_maps, emb = _make_in_maps(inputs)
    B = N_CORES * B_LOC

    res = run_bass_kernel_spmd(nc, in_maps, core_ids=list(range(N_CORES)))

    z_parts = []
    idx_parts = []
    for c in range(N_CORES):
        r = res.results[c]
        z_parts.append(r["z_out"])                      # [4, 64, 511]
        arr = r["idx_out"]                              # [128, 16]
        loc = np.empty(B_LOC * T2, dtype=np.int64)
        for b in range(B_LOC):
            for ch in range(4):
                c0 = ch * 128
                cs = min(128, T2 - c0)
                loc[b * T2 + c0: b * T2 + c0 + cs] = arr[:cs, b * 4 + ch]
        idx_parts.append(loc)

    z = np.concatenate(z_parts, axis=0).transpose(0, 2, 1)   # [32, 511, 64] f32
    z = np.ascontiguousarray(z)
    idx = np.concatenate(idx_parts)                          # [16352]

    q = emb[idx].reshape(B, T2, D)
    # straight-through output, same fp32 association as the reference
    q_st = z + (q - z)
    diff = z.astype(np.float64) - q.astype(np.float64)
    loss = np.float32(0.25 * np.mean(diff * diff))
    counts = np.bincount(idx, minlength=M).astype(np.float64)
    avg = counts / idx.shape[0]
    perplexity = np.float32(np.exp(-np.sum(avg * np.log(avg + 1e-10))))
    return q_st, loss, perplexity
